# revision 13
# baseline (speedup 1.0000x reference)
"""Bass/Trainium2 kernel for nn_Attention_28140625723842 (v2).

Multi-head attention (B=2, S=2048, D=1024, H=16, DH=64) with key-padding
mask, sharded over 8 NeuronCores as 2 batches x 4 head-groups.

v2 design (vs the v1 baseline at ~150-162us):
  - Q/K projections run in fp8(e4m3) DoubleRow perf mode: contraction
    pairs (d, d+128) packed per PE cell -> ~1.8x matmul throughput and
    half the activation DMA bytes. V projection stays bf16 (its error
    feeds the output directly).
  - k/v tokens host-compacted to the unmasked set (padded to 128).
  - Attention runs as 4 blocks (hp head-pair x qb half-of-S). Per k-tile
    the two heads of the pair are two phase-offset streams (psA/psB,
    each single-buffered): while ScalarE exps stream A then B, the PE
    refills the other bank -> ScalarE (the wall at ~75us) stays
    saturated.
  - P@V for block n is woven into block n+1's k-tile loop (PSUM has
    exactly 8 banks: 2+2 for scores + 4 for the deferred accumulators).
    Block 0's weave slots instead run the V projection.
  - The V matrix carries a ones column, so the softmax denominator falls
    out of the P@V matmul as row DH.
  - Normalize tail without PE transposes: PV accumulator -> bf16 SBUF
    copy -> DMA-engine transpose [96,1024]->[128,8,96] -> one batched
    DVE reciprocal + one broadcasted tensor_tensor multiply per head.
  - Output bf16, DMA'd per block as soon as it is normalized; host
    upcasts to fp32.
"""

import numpy as np

B, S, D, H = 2, 2048, 1024, 16
DH = D // H            # 64 head dim
NCORES = 8
GROUPS = NCORES // B   # 4 head groups
HL = H // GROUPS       # 4 heads per core
GW = HL * DH           # 256 output columns per core

P = 128
ND = D // P            # 8 contraction tiles (bf16 path)
ND2 = D // 256         # 4 DoubleRow contraction tiles (fp8 path)
NT = S // P            # 16 q token tiles
QB = 1024              # q block (one exp op width)
NQB = S // QB          # 2
NQ8 = QB // P          # 8 q token tiles per block
CH = 512               # matmul free-dim chunk (one PSUM bank fp32)
NCH = QB // CH         # 2

_CACHE = {}


def _chunks(total, width):
    out = []
    o = 0
    while o < total:
        w = min(width, total - o)
        out.append((o, w))
        o += w
    return out


def _build_nc(nk, use_bias=False):
    import concourse.bacc as bacc
    import concourse.mybir as mybir
    import concourse.tile as tile
    from concourse.bass import broadcast_tensor_aps

    f32 = mybir.dt.float32
    bf16 = mybir.dt.bfloat16
    fp8 = mybir.dt.float8e4
    i32 = mybir.dt.int32
    Exp = mybir.ActivationFunctionType.Exp
    DR = mybir.MatmulPerfMode.DoubleRow
    SCALE = float(1.0 / np.sqrt(np.float32(D)))
    NTK = nk // P          # k token tiles (compacted)

    nc = bacc.Bacc(None, target_bir_lowering=False)
    qx_d = nc.dram_tensor("qx", [P, ND2, 2, S], fp8, kind="ExternalInput")
    kx_d = nc.dram_tensor("kx", [P, ND2, 2, nk], fp8, kind="ExternalInput")
    vx_d = nc.dram_tensor("vx", [P, ND, nk], bf16, kind="ExternalInput")
    wq_d = nc.dram_tensor("wq", [P, ND2, 2, GW], fp8, kind="ExternalInput")
    wk_d = nc.dram_tensor("wk", [P, ND2, 2, GW], fp8, kind="ExternalInput")
    wv_d = nc.dram_tensor("wv", [P, ND, GW], bf16, kind="ExternalInput")
    mask_d = nc.dram_tensor("mask", [nk], i32, kind="ExternalInput")
    out_d = nc.dram_tensor("out", [S, GW], bf16, kind="ExternalOutput")
    if use_bias:
        bq_d = nc.dram_tensor("bq", [GW], bf16, kind="ExternalInput")
        bk_d = nc.dram_tensor("bk", [GW], bf16, kind="ExternalInput")
        bv_d = nc.dram_tensor("bv", [GW], bf16, kind="ExternalInput")

    with tile.TileContext(nc) as tc:
        with (
            tc.tile_pool(name="consts", bufs=1) as consts,
            tc.tile_pool(name="persist", bufs=1) as persist,
            tc.tile_pool(name="exps", bufs=24) as expp,
            tc.tile_pool(name="pvsb", bufs=4) as pvsbp,
            tc.tile_pool(name="tpsb", bufs=4) as tpsbp,
            tc.tile_pool(name="recs", bufs=4) as recsp,
        ):
            # mask[k] -> per-partition exp bias: (m - 1) * 1e9  (0 or -1e9)
            maski = consts.tile([P, NTK], i32, tag="maski")
            nc.scalar.dma_start(maski, mask_d.rearrange("(t p) -> p t", p=P))
            maskb = consts.tile([P, NTK], f32, tag="maskb")
            nc.vector.tensor_scalar(
                maskb, maski, -1.0, 1e9,
                mybir.AluOpType.add, mybir.AluOpType.mult,
            )
            # tiny dummy exp to pull the ~1.3us ACT_TABLE_LOAD off the
            # critical path (runs during the input-DMA ramp)
            warm = consts.tile([1, 1], f32, tag="warm")
            nc.scalar.activation(warm, maskb[0:1, 0:1], Exp)

            brow = {}
            if use_bias:
                ones = consts.tile([1, CH], bf16, tag="ones")
                nc.vector.memset(ones, 1.0)
                for nm, drm in (("q", bq_d), ("k", bk_d), ("v", bv_d)):
                    t = consts.tile([1, GW], bf16, tag=f"bias_{nm}")
                    nc.scalar.dma_start(t, drm[None, :])
                    brow[nm] = t

            # weights first (small, unblock first matmuls)
            wk_sb = persist.tile([P, ND2, 2, GW], fp8, tag="wk")
            nc.scalar.dma_start(wk_sb, wk_d[:, :, :, :])
            wq_sb = persist.tile([P, ND2, 2, GW], fp8, tag="wq")
            nc.scalar.dma_start(wq_sb, wq_d[:, :, :, :])
            wv_sb = persist.tile([P, ND, GW], bf16, tag="wv")
            nc.scalar.dma_start(wv_sb, wv_d[:, :, :])

            # k activations on the sync ring, chunked by (dt2, nk-half)
            kx_sb = persist.tile([P, ND2, 2, nk], fp8, tag="kx")
            nkh = (NTK // 2) * P
            for d2 in range(ND2):
                nc.sync.dma_start(kx_sb[:, d2, :, :nkh],
                                  kx_d[:, d2, :, :nkh])
            for d2 in range(ND2):
                nc.sync.dma_start(kx_sb[:, d2, :, nkh:],
                                  kx_d[:, d2, :, nkh:])
            qx_sb = persist.tile([P, ND2, 2, S], fp8, tag="qx")
            for qb in range(NQB):
                qs = slice(qb * QB, (qb + 1) * QB)
                for d2 in range(ND2):
                    nc.sync.dma_start(qx_sb[:, d2, :, qs], qx_d[:, d2, :, qs])
            # v activations on the scalar ring (parallel to qx)
            vx_sb = persist.tile([P, ND, nk], bf16, tag="vx")
            nc.scalar.dma_start(vx_sb, vx_d[:, :, :])

            QT = persist.tile([P, 2, S], bf16, tag="QT")
            KT = persist.tile([P, 2, nk], bf16, tag="KT")
            V = persist.tile([P, NTK, HL * (DH + 1)], bf16, tag="V")
            V4 = V.rearrange("p t (h e) -> p t h e", h=HL)
            out_sb = persist.tile([P, NT, GW], bf16, tag="osb")
            out_blk = out_d.rearrange("(t p) w -> p t w", p=P)

            for h in range(HL):
                nc.vector.memset(V4[:, :, h, DH], 1.0)

            with (
                tc.tile_pool(name="pssA", bufs=1, space="PSUM") as pssA,
                tc.tile_pool(name="pssB", bufs=1, space="PSUM") as pssB,
            ):
                def proj_qk_group(which, x_sb, w_sb, OUT, hp, co, cw):
                    # one fp8 DoubleRow projection group: OUT[:, hp, co:co+cw]
                    pt = pp.tile([P, CH], f32, tag="pp")
                    for d2 in range(ND2):
                        nc.tensor.matmul(
                            pt[:, :cw],
                            lhsT=w_sb[:, d2, :, hp * P:(hp + 1) * P],
                            rhs=x_sb[:, d2, :, co:co + cw],
                            start=(d2 == 0),
                            stop=(not use_bias and d2 == ND2 - 1),
                            perf_mode=DR,
                        )
                    if use_bias:
                        nc.tensor.matmul(
                            pt[:, :cw],
                            lhsT=brow[which][:, hp * P:(hp + 1) * P],
                            rhs=ones[:, :cw],
                            start=False, stop=True,
                        )
                    nc.vector.tensor_copy(
                        out=OUT[:, hp, co:co + cw], in_=pt[:, :cw]
                    )

                def emit_vproj(tt):
                    # one V-projection token tile (bf16), woven into block 0
                    vp = pp.tile([P, CH], f32, tag="pp")
                    for dt_ in range(ND):
                        nc.tensor.matmul(
                            vp[:, :GW],
                            lhsT=vx_sb[:, dt_, tt * P:(tt + 1) * P],
                            rhs=wv_sb[:, dt_, :],
                            start=(dt_ == 0),
                            stop=(not use_bias and dt_ == ND - 1),
                        )
                    if use_bias:
                        nc.tensor.matmul(
                            vp[:, :GW],
                            lhsT=ones[:, :P],
                            rhs=brow["v"],
                            start=False, stop=True,
                        )
                    nc.vector.tensor_copy(
                        out=V4[:, tt, :, :DH],
                        in_=vp[:, :GW].rearrange("p (h e) -> p h e", h=HL),
                    )

                def emit_block(hp, qb, weave_pv, weave_v, extra_weave=None):
                    # one attention block: two phase-offset exp streams.
                    # weave_pv: (pvts, ets, ...) of the PREVIOUS block whose
                    # P@V matmuls ride this block's k-tile loop.
                    # Weave work is emitted BEFORE the unit's scores so a
                    # stalled score matmul (WAR on its exp) never head-of-line
                    # blocks the weave, and the weave never delays the next
                    # score dispatch past the exp window.
                    ets = ([], [])
                    for kt in range(NTK):
                        if weave_pv is not None:
                            emit_pv_unit(weave_pv, kt)
                        if weave_v:
                            emit_vproj(kt)
                        if extra_weave and kt >= 4:
                            extra_weave.pop(0)()
                        for st, pool in ((0, pssA), (1, pssB)):
                            ps = pool.tile([P, QB], f32, tag=f"s{st}")
                            po = st * DH
                            for c in range(NCH):
                                nc.tensor.matmul(
                                    ps[:, c * CH:(c + 1) * CH],
                                    lhsT=KT[po:po + DH, hp,
                                            kt * P:(kt + 1) * P],
                                    rhs=QT[po:po + DH, hp,
                                           qb * QB + c * CH:
                                           qb * QB + (c + 1) * CH],
                                    start=True, stop=True,
                                )
                            e = expp.tile([P, QB], bf16, tag="e")
                            nc.scalar.activation(
                                e, ps, Exp,
                                bias=maskb[:, kt:kt + 1], scale=SCALE,
                            )
                            ets[st].append(e)
                    return ets

                def emit_pv_unit(pv_state, kt):
                    pvts, p_ets, _, _ = pv_state
                    for st in range(2):
                        for c in range(NCH):
                            nc.tensor.matmul(
                                pvts[st][:, c * CH:(c + 1) * CH],
                                lhsT=V[:, kt,
                                       (2 * pv_state[2] + st) * (DH + 1):
                                       (2 * pv_state[2] + st + 1) * (DH + 1)],
                                rhs=p_ets[st][kt][:, c * CH:(c + 1) * CH],
                                start=(kt == 0), stop=(kt == NTK - 1),
                            )

                def emit_normalize(pv_state):
                    # prev block's accumulators -> bf16 -> DMA transpose ->
                    # batched reciprocal -> broadcast multiply -> out DMA
                    pvts, _, hp_p, qb_p = pv_state
                    for st in range(2):
                        pv_sb = pvsbp.tile([96, QB], bf16, tag="pvsb")
                        nc.vector.tensor_copy(
                            out=pv_sb[:DH + 1, :], in_=pvts[st]
                        )
                        tps = tpsbp.tile([P, NQ8, 96], bf16, tag="tps")
                        nc.sync.dma_start_transpose(tps, pv_sb[:, :])
                        rec = recsp.tile([P, NQ8, 1], f32, tag="rec")
                        nc.vector.reciprocal(rec, tps[:, :, DH:DH + 1])
                        col = hp_p * P + st * DH
                        o_ap = out_sb[:, qb_p * NQ8:(qb_p + 1) * NQ8,
                                      col:col + DH]
                        in0 = tps[:, :, :DH]
                        in0b, in1b = broadcast_tensor_aps(in0, rec)
                        nc.vector.tensor_tensor(
                            out=o_ap, in0=in0b, in1=in1b,
                            op=mybir.AluOpType.mult,
                        )
                        nc.sync.dma_start(
                            out_blk[:, qb_p * NQ8:(qb_p + 1) * NQ8,
                                    col:col + DH],
                            out_sb[:, qb_p * NQ8:(qb_p + 1) * NQ8,
                                   col:col + DH],
                        )

                blocks = [(0, 0), (0, 1), (1, 0), (1, 1)]

                with tc.tile_pool(name="pp", bufs=2, space="PSUM") as pp:
                    # upfront projections (fp8 DoubleRow), hp=0 first: block 0
                    # only needs the hp=0 halves of KT and QT[qb=0]
                    for hp in range(2):
                        for co, cw in _chunks(nk, CH):
                            proj_qk_group("k", kx_sb, wk_sb, KT, hp, co, cw)
                        for c in range(NCH):
                            proj_qk_group("q", qx_sb, wq_sb, QT,
                                          hp, c * CH, CH)
                    # qb=1 Q-projection groups ride block 0's later units
                    # (their qx DMA lands last); V projection rides every unit
                    qb1 = [
                        (lambda hp=hp, c=c: proj_qk_group(
                            "q", qx_sb, wq_sb, QT, hp, QB + c * CH, CH))
                        for hp in range(2) for c in range(NCH)
                    ]
                    ets0 = emit_block(*blocks[0], weave_pv=None, weave_v=True,
                                      extra_weave=qb1)

                with tc.tile_pool(name="pvt", bufs=2, space="PSUM") as pvtp:
                    prev = None
                    prev_ets = ets0
                    prev_blk = blocks[0]
                    for bi in range(1, 4):
                        hp, qb = blocks[bi]
                        pvts = [
                            pvtp.tile([DH + 1, QB], f32, tag="pvt",
                                      name=f"pvt_{bi}_{st}")
                            for st in range(2)
                        ]
                        pv_state = (pvts, prev_ets, prev_blk[0], prev_blk[1])
                        ets = emit_block(hp, qb, weave_pv=pv_state,
                                         weave_v=False)
                        emit_normalize(pv_state)
                        prev_ets = ets
                        prev_blk = blocks[bi]
                    # tail: last block's P@V + normalize
                    pvts = [
                        pvtp.tile([DH + 1, QB], f32, tag="pvt",
                                  name=f"pvt_tail_{st}")
                        for st in range(2)
                    ]
                    pv_state = (pvts, prev_ets, prev_blk[0], prev_blk[1])
                    for kt in range(NTK):
                        emit_pv_unit(pv_state, kt)
                    emit_normalize(pv_state)
    nc.compile()
    return nc


def _get_nc(nk, use_bias=False):
    key = (nk, use_bias)
    if key not in _CACHE:
        _CACHE[key] = _build_nc(nk, use_bias=use_bias)
    return _CACHE[key]


def _run(nc, in_maps, trace=False):
    from concourse.bass_utils import run_bass_kernel_spmd

    return run_bass_kernel_spmd(
        nc, in_maps, core_ids=list(range(NCORES)), trace=trace
    )


def _make_in_maps(q, k, v, mask, Wq, bq, Wk, bk, Wv, bv):
    import ml_dtypes

    bf16 = ml_dtypes.bfloat16
    fp8 = ml_dtypes.float8_e4m3fn
    q = np.asarray(q, np.float32)
    k = np.asarray(k, np.float32)
    v = np.asarray(v, np.float32)
    mask = np.asarray(mask, np.int32)
    Wq = np.asarray(Wq, np.float32)
    Wk = np.asarray(Wk, np.float32)
    Wv = np.asarray(Wv, np.float32)

    use_bias = bool(
        np.any(np.asarray(bq, np.float32))
        or np.any(np.asarray(bk, np.float32))
        or np.any(np.asarray(bv, np.float32))
    )

    idxs = [np.nonzero(mask[b])[0] for b in range(B)]
    neff = max(1, max(len(ix) for ix in idxs))
    nk = -(-neff // P) * P  # round up to multiple of 128

    def pair4(x):  # [D, w] -> [P, ND2, 2, w] fp8 (d = d2*256 + ko*128 + p)
        w = x.shape[1]
        return np.ascontiguousarray(
            x.reshape(ND2, 2, P, w).transpose(2, 0, 1, 3)
        ).astype(fp8)

    def tile8(x):  # [D, w] -> [P, ND, w]
        w = x.shape[1]
        return np.ascontiguousarray(x.reshape(ND, P, w).transpose(1, 0, 2))

    qxs, kxs, vxs, mks = [], [], [], []
    for b in range(B):
        ix = idxs[b]
        qxs.append(pair4(q[b].T))
        kc = np.zeros((D, nk), np.float32)
        vc = np.zeros((D, nk), np.float32)
        kc[:, :len(ix)] = k[b].T[:, ix]
        vc[:, :len(ix)] = v[b].T[:, ix]
        kxs.append(pair4(kc))
        vxs.append(tile8(vc).astype(bf16))
        m = np.zeros((nk,), np.int32)
        m[:len(ix)] = 1
        mks.append(m)

    in_maps = []
    for c in range(NCORES):
        b, g = divmod(c, GROUPS)
        cols = slice(g * GW, (g + 1) * GW)
        im = {
            "qx": qxs[b],
            "kx": kxs[b],
            "vx": vxs[b],
            "wq": pair4(Wq[:, cols]),
            "wk": pair4(Wk[:, cols]),
            "wv": tile8(Wv[:, cols]).astype(bf16),
            "mask": mks[b],
        }
        if use_bias:
            im["bq"] = np.ascontiguousarray(bq[cols]).astype(bf16)
            im["bk"] = np.ascontiguousarray(bk[cols]).astype(bf16)
            im["bv"] = np.ascontiguousarray(bv[cols]).astype(bf16)
        in_maps.append(im)
    return nk, use_bias, in_maps


def _assemble(results):
    out = np.empty((B, S, D), np.float32)
    for c in range(NCORES):
        b, g = divmod(c, GROUPS)
        out[b, :, g * GW:(g + 1) * GW] = results[c]["out"].astype(np.float32)
    return out


def kernel(q, k, v, mask, Wq, bq, Wk, bk, Wv, bv):
    nk, use_bias, in_maps = _make_in_maps(q, k, v, mask, Wq, bq, Wk, bk, Wv, bv)
    res = _run(_get_nc(nk, use_bias), in_maps, trace=False)
    return _assemble(res.results)


def _install_ntff_hook():
    """The image's antenv stub lacks axon_hooks; synthesize it and register
    the ctypes NTFF hook that trn_agent_boot would have installed."""
    import sys
    import types

    import antenv

    if "antenv.axon_hooks" in sys.modules:
        return
    mod = types.ModuleType("antenv.axon_hooks")
    state = {"hook": None}
    mod.set_axon_ntff_profile_hook = lambda h: state.__setitem__("hook", h)
    mod.get_axon_ntff_profile_hook = lambda: state["hook"]
    sys.modules["antenv.axon_hooks"] = mod
    antenv.axon_hooks = mod
    try:
        from trn_agent_boot.trn_boot import _ntff_profile_via_ctypes

        mod.set_axon_ntff_profile_hook(
            _ntff_profile_via_ctypes("/opt/axon/libaxon_pjrt.so")
        )
    except Exception as e:
        print(f"ntff hook registration failed: {e}")


def _exec_ns_from_newest_ntff():
    """Span of the newest NTFF json's DMA events — matches gauge's
    first/last-useful exec time when instruction events are absent."""
    import glob
    import json as _json
    import os

    try:
        path = max(glob.glob("/tmp/tmp*/ntff_0.json"), key=os.path.getmtime)
        d = _json.load(open(path))
        ev = d.get("dma", [])
        if not ev:
            return None
        t0 = min(e["timestamp"] for e in ev)
        t1 = max(e["timestamp"] + e.get("duration", 0) for e in ev)
        return t1 - t0
    except Exception:
        return None


def kernel_traced(q, k, v, mask, Wq, bq, Wk, bk, Wv, bv):
    """Same as kernel() but also returns (output, exec_time_ns)."""
    _install_ntff_hook()
    nk, use_bias, in_maps = _make_in_maps(q, k, v, mask, Wq, bq, Wk, bk, Wv, bv)
    nc = _get_nc(nk, use_bias)
    try:
        res = _run(nc, in_maps, trace=True)
        return _assemble(res.results), res.exec_time_ns
    except Exception:
        # gauge's NTFF->perfetto step can fail on kernels whose profile
        # lacks instruction events (`assert insts`); the NTFF json still
        # exists, so recover the exec time from its DMA span and rerun
        # untraced for the outputs.
        exec_ns = _exec_ns_from_newest_ntff()
        res = _run(nc, in_maps, trace=False)
        return _assemble(res.results), exec_ns


# revision 14
# speedup vs baseline: 1.1533x; 1.1533x over previous
"""Bass/Trainium2 kernel for nn_Attention_28140625723842 (v2).

Multi-head attention (B=2, S=2048, D=1024, H=16, DH=64) with key-padding
mask, sharded over 8 NeuronCores as 2 batches x 4 head-groups.

v2 design (vs the v1 baseline at ~150-162us):
  - Q/K projections run in fp8(e4m3) DoubleRow perf mode: contraction
    pairs (d, d+128) packed per PE cell -> ~1.8x matmul throughput and
    half the activation DMA bytes. V projection stays bf16 (its error
    feeds the output directly).
  - k/v tokens host-compacted to the unmasked set (padded to 128).
  - Attention runs as 4 blocks (hp head-pair x qb half-of-S). Per k-tile
    the two heads of the pair are two phase-offset streams (psA/psB,
    each single-buffered): while ScalarE exps stream A then B, the PE
    refills the other bank -> ScalarE (the wall at ~75us) stays
    saturated.
  - P@V for block n is woven into block n+1's k-tile loop (PSUM has
    exactly 8 banks: 2+2 for scores + 4 for the deferred accumulators).
    Block 0's weave slots instead run the V projection.
  - The V matrix carries a ones column, so the softmax denominator falls
    out of the P@V matmul as row DH.
  - Normalize tail without PE transposes: PV accumulator -> bf16 SBUF
    copy -> DMA-engine transpose [96,1024]->[128,8,96] -> one batched
    DVE reciprocal + one broadcasted tensor_tensor multiply per head.
  - Output bf16, DMA'd per block as soon as it is normalized; host
    upcasts to fp32.
"""

import numpy as np

B, S, D, H = 2, 2048, 1024, 16
DH = D // H            # 64 head dim
NCORES = 8
GROUPS = NCORES // B   # 4 head groups
HL = H // GROUPS       # 4 heads per core
GW = HL * DH           # 256 output columns per core

P = 128
ND = D // P            # 8 contraction tiles (bf16 path)
ND2 = D // 256         # 4 DoubleRow contraction tiles (fp8 path)
NT = S // P            # 16 q token tiles
QB = 1024              # q block (one exp op width)
NQB = S // QB          # 2
NQ8 = QB // P          # 8 q token tiles per block
CH = 512               # matmul free-dim chunk (one PSUM bank fp32)
NCH = QB // CH         # 2

_CACHE = {}


def _chunks(total, width):
    out = []
    o = 0
    while o < total:
        w = min(width, total - o)
        out.append((o, w))
        o += w
    return out


def _build_nc(nk, use_bias=False):
    import concourse.bacc as bacc
    import concourse.mybir as mybir
    import concourse.tile as tile
    from concourse.bass import broadcast_tensor_aps

    f32 = mybir.dt.float32
    bf16 = mybir.dt.bfloat16
    fp8 = mybir.dt.float8e4
    i32 = mybir.dt.int32
    Exp = mybir.ActivationFunctionType.Exp
    DR = mybir.MatmulPerfMode.DoubleRow
    SCALE = float(1.0 / np.sqrt(np.float32(D)))
    NTK = nk // P          # k token tiles (compacted)

    nc = bacc.Bacc(None, target_bir_lowering=False)
    qx_d = nc.dram_tensor("qx", [P, ND2, 2, S], fp8, kind="ExternalInput")
    kx_d = nc.dram_tensor("kx", [P, ND2, 2, nk], fp8, kind="ExternalInput")
    vx_d = nc.dram_tensor("vx", [P, ND, nk], bf16, kind="ExternalInput")
    wq_d = nc.dram_tensor("wq", [P, ND2, 2, GW], fp8, kind="ExternalInput")
    wk_d = nc.dram_tensor("wk", [P, ND2, 2, GW], fp8, kind="ExternalInput")
    wv_d = nc.dram_tensor("wv", [P, ND, GW], bf16, kind="ExternalInput")
    mask_d = nc.dram_tensor("mask", [nk], i32, kind="ExternalInput")
    out_d = nc.dram_tensor("out", [S, GW], bf16, kind="ExternalOutput")
    if use_bias:
        bq_d = nc.dram_tensor("bq", [GW], bf16, kind="ExternalInput")
        bk_d = nc.dram_tensor("bk", [GW], bf16, kind="ExternalInput")
        bv_d = nc.dram_tensor("bv", [GW], bf16, kind="ExternalInput")

    with tile.TileContext(nc) as tc:
        with (
            tc.tile_pool(name="consts", bufs=1) as consts,
            tc.tile_pool(name="persist", bufs=1) as persist,
            tc.tile_pool(name="exps", bufs=24) as expp,
            tc.tile_pool(name="pvsb", bufs=4) as pvsbp,
            tc.tile_pool(name="tpsb", bufs=4) as tpsbp,
            tc.tile_pool(name="recs", bufs=4) as recsp,
        ):
            # mask[k] -> per-partition exp bias: (m - 1) * 1e9  (0 or -1e9)
            maski = consts.tile([P, NTK], i32, tag="maski")
            nc.scalar.dma_start(maski, mask_d.rearrange("(t p) -> p t", p=P))
            maskb = consts.tile([P, NTK], f32, tag="maskb")
            nc.vector.tensor_scalar(
                maskb, maski, -1.0, 1e9,
                mybir.AluOpType.add, mybir.AluOpType.mult,
            )
            # tiny dummy exp to pull the ~1.3us ACT_TABLE_LOAD off the
            # critical path (runs during the input-DMA ramp)
            warm = consts.tile([1, 1], f32, tag="warm")
            nc.scalar.activation(warm, maskb[0:1, 0:1], Exp)

            brow = {}
            if use_bias:
                ones = consts.tile([1, CH], bf16, tag="ones")
                nc.vector.memset(ones, 1.0)
                for nm, drm in (("q", bq_d), ("k", bk_d), ("v", bv_d)):
                    t = consts.tile([1, GW], bf16, tag=f"bias_{nm}")
                    nc.scalar.dma_start(t, drm[None, :])
                    brow[nm] = t

            # weights first (small, unblock first matmuls)
            wk_sb = persist.tile([P, ND2, 2, GW], fp8, tag="wk")
            nc.scalar.dma_start(wk_sb, wk_d[:, :, :, :])
            wq_sb = persist.tile([P, ND2, 2, GW], fp8, tag="wq")
            nc.scalar.dma_start(wq_sb, wq_d[:, :, :, :])
            wv_sb = persist.tile([P, ND, GW], bf16, tag="wv")
            nc.scalar.dma_start(wv_sb, wv_d[:, :, :])

            # k activations on the sync ring, chunked by (dt2, nk-half)
            kx_sb = persist.tile([P, ND2, 2, nk], fp8, tag="kx")
            nkh = (NTK // 2) * P
            for d2 in range(ND2):
                nc.sync.dma_start(kx_sb[:, d2, :, :nkh],
                                  kx_d[:, d2, :, :nkh])
            for d2 in range(ND2):
                nc.sync.dma_start(kx_sb[:, d2, :, nkh:],
                                  kx_d[:, d2, :, nkh:])
            qx_sb = persist.tile([P, ND2, 2, S], fp8, tag="qx")
            for qb in range(NQB):
                qs = slice(qb * QB, (qb + 1) * QB)
                for d2 in range(ND2):
                    nc.sync.dma_start(qx_sb[:, d2, :, qs], qx_d[:, d2, :, qs])
            # v activations on the scalar ring (parallel to qx)
            vx_sb = persist.tile([P, ND, nk], bf16, tag="vx")
            nc.scalar.dma_start(vx_sb, vx_d[:, :, :])

            QT = persist.tile([P, 2, S], bf16, tag="QT")
            KT = persist.tile([P, 2, nk], bf16, tag="KT")
            V = persist.tile([P, NTK, HL * (DH + 1)], bf16, tag="V")
            V4 = V.rearrange("p t (h e) -> p t h e", h=HL)
            out_sb = persist.tile([P, NT, GW], bf16, tag="osb")
            out_blk = out_d.rearrange("(t p) w -> p t w", p=P)

            for h in range(HL):
                nc.vector.memset(V4[:, :, h, DH], 1.0)

            with (
                tc.tile_pool(name="pssA", bufs=1, space="PSUM") as pssA,
                tc.tile_pool(name="pssB", bufs=1, space="PSUM") as pssB,
            ):
                def proj_qk_group(which, x_sb, w_sb, OUT, hp, co, cw):
                    # one fp8 DoubleRow projection group: OUT[:, hp, co:co+cw]
                    pt = pp.tile([P, CH], f32, tag="pp")
                    for d2 in range(ND2):
                        nc.tensor.matmul(
                            pt[:, :cw],
                            lhsT=w_sb[:, d2, :, hp * P:(hp + 1) * P],
                            rhs=x_sb[:, d2, :, co:co + cw],
                            start=(d2 == 0),
                            stop=(not use_bias and d2 == ND2 - 1),
                            perf_mode=DR,
                        )
                    if use_bias:
                        nc.tensor.matmul(
                            pt[:, :cw],
                            lhsT=brow[which][:, hp * P:(hp + 1) * P],
                            rhs=ones[:, :cw],
                            start=False, stop=True,
                        )
                    nc.vector.tensor_copy(
                        out=OUT[:, hp, co:co + cw], in_=pt[:, :cw]
                    )

                def emit_vproj(tt):
                    # one V-projection token tile (bf16), woven into block 0
                    vp = pp.tile([P, CH], f32, tag="pp")
                    for dt_ in range(ND):
                        nc.tensor.matmul(
                            vp[:, :GW],
                            lhsT=vx_sb[:, dt_, tt * P:(tt + 1) * P],
                            rhs=wv_sb[:, dt_, :],
                            start=(dt_ == 0),
                            stop=(not use_bias and dt_ == ND - 1),
                        )
                    if use_bias:
                        nc.tensor.matmul(
                            vp[:, :GW],
                            lhsT=ones[:, :P],
                            rhs=brow["v"],
                            start=False, stop=True,
                        )
                    nc.vector.tensor_copy(
                        out=V4[:, tt, :, :DH],
                        in_=vp[:, :GW].rearrange("p (h e) -> p h e", h=HL),
                    )

                def emit_block(hp, qb, weave_pv, weave_v, extra_weave=None):
                    # one attention block: two phase-offset exp streams.
                    # weave_pv: (pvts, ets, ...) of the PREVIOUS block whose
                    # P@V matmuls ride this block's k-tile loop.
                    # Weave work is emitted BEFORE the unit's scores so a
                    # stalled score matmul (WAR on its exp) never head-of-line
                    # blocks the weave, and the weave never delays the next
                    # score dispatch past the exp window.
                    ets = ([], [])
                    for kt in range(NTK):
                        if weave_pv is not None:
                            emit_pv_unit(weave_pv, kt)
                        if weave_v:
                            emit_vproj(kt)
                        if extra_weave and kt >= 4:
                            extra_weave.pop(0)()
                        for st, pool in ((0, pssA), (1, pssB)):
                            ps = pool.tile([P, QB], f32, tag=f"s{st}")
                            po = st * DH
                            for c in range(NCH):
                                nc.tensor.matmul(
                                    ps[:, c * CH:(c + 1) * CH],
                                    lhsT=KT[po:po + DH, hp,
                                            kt * P:(kt + 1) * P],
                                    rhs=QT[po:po + DH, hp,
                                           qb * QB + c * CH:
                                           qb * QB + (c + 1) * CH],
                                    start=True, stop=True,
                                )
                            e = expp.tile([P, QB], bf16, tag="e")
                            nc.scalar.activation(
                                e, ps, Exp,
                                bias=maskb[:, kt:kt + 1], scale=SCALE,
                            )
                            ets[st].append(e)
                    return ets

                def emit_pv_unit(pv_state, kt):
                    pvts, p_ets, _, _ = pv_state
                    for st in range(2):
                        for c in range(NCH):
                            nc.tensor.matmul(
                                pvts[st][:, c * CH:(c + 1) * CH],
                                lhsT=V[:, kt,
                                       (2 * pv_state[2] + st) * (DH + 1):
                                       (2 * pv_state[2] + st + 1) * (DH + 1)],
                                rhs=p_ets[st][kt][:, c * CH:(c + 1) * CH],
                                start=(kt == 0), stop=(kt == NTK - 1),
                            )

                def emit_normalize(pv_state):
                    # prev block's accumulators -> bf16 -> DMA transpose ->
                    # batched reciprocal -> broadcast multiply -> out DMA
                    pvts, _, hp_p, qb_p = pv_state
                    for st in range(2):
                        pv_sb = pvsbp.tile([96, QB], bf16, tag="pvsb")
                        nc.vector.tensor_copy(
                            out=pv_sb[:DH + 1, :], in_=pvts[st]
                        )
                        tps = tpsbp.tile([P, NQ8, 96], bf16, tag="tps")
                        nc.sync.dma_start_transpose(tps, pv_sb[:, :])
                        rec = recsp.tile([P, NQ8, 1], f32, tag="rec")
                        nc.vector.reciprocal(rec, tps[:, :, DH:DH + 1])
                        col = hp_p * P + st * DH
                        o_ap = out_sb[:, qb_p * NQ8:(qb_p + 1) * NQ8,
                                      col:col + DH]
                        in0 = tps[:, :, :DH]
                        in0b, in1b = broadcast_tensor_aps(in0, rec)
                        nc.vector.tensor_tensor(
                            out=o_ap, in0=in0b, in1=in1b,
                            op=mybir.AluOpType.mult,
                        )
                    nc.sync.dma_start(
                        out_blk[:, qb_p * NQ8:(qb_p + 1) * NQ8,
                                hp_p * P:(hp_p + 1) * P],
                        out_sb[:, qb_p * NQ8:(qb_p + 1) * NQ8,
                               hp_p * P:(hp_p + 1) * P],
                    )

                blocks = [(0, 0), (0, 1), (1, 0), (1, 1)]

                with tc.tile_pool(name="pp", bufs=2, space="PSUM") as pp:
                    # upfront projections (fp8 DoubleRow), hp=0 first: block 0
                    # only needs the hp=0 halves of KT and QT[qb=0]
                    for hp in range(2):
                        for co, cw in _chunks(nk, CH):
                            proj_qk_group("k", kx_sb, wk_sb, KT, hp, co, cw)
                        for c in range(NCH):
                            proj_qk_group("q", qx_sb, wq_sb, QT,
                                          hp, c * CH, CH)
                    # qb=1 Q-projection groups ride block 0's later units
                    # (their qx DMA lands last); V projection rides every unit
                    qb1 = [
                        (lambda hp=hp, c=c: proj_qk_group(
                            "q", qx_sb, wq_sb, QT, hp, QB + c * CH, CH))
                        for hp in range(2) for c in range(NCH)
                    ]
                    ets0 = emit_block(*blocks[0], weave_pv=None, weave_v=True,
                                      extra_weave=qb1)

                with tc.tile_pool(name="pvt", bufs=2, space="PSUM") as pvtp:
                    prev = None
                    prev_ets = ets0
                    prev_blk = blocks[0]
                    for bi in range(1, 4):
                        hp, qb = blocks[bi]
                        pvts = [
                            pvtp.tile([DH + 1, QB], f32, tag="pvt",
                                      name=f"pvt_{bi}_{st}")
                            for st in range(2)
                        ]
                        pv_state = (pvts, prev_ets, prev_blk[0], prev_blk[1])
                        ets = emit_block(hp, qb, weave_pv=pv_state,
                                         weave_v=False)
                        emit_normalize(pv_state)
                        prev_ets = ets
                        prev_blk = blocks[bi]
                    # tail: last block's P@V + normalize
                    pvts = [
                        pvtp.tile([DH + 1, QB], f32, tag="pvt",
                                  name=f"pvt_tail_{st}")
                        for st in range(2)
                    ]
                    pv_state = (pvts, prev_ets, prev_blk[0], prev_blk[1])
                    for kt in range(NTK):
                        emit_pv_unit(pv_state, kt)
                    emit_normalize(pv_state)
    nc.compile()
    return nc


def _get_nc(nk, use_bias=False):
    key = (nk, use_bias)
    if key not in _CACHE:
        _CACHE[key] = _build_nc(nk, use_bias=use_bias)
    return _CACHE[key]


def _run(nc, in_maps, trace=False):
    from concourse.bass_utils import run_bass_kernel_spmd

    return run_bass_kernel_spmd(
        nc, in_maps, core_ids=list(range(NCORES)), trace=trace
    )


def _make_in_maps(q, k, v, mask, Wq, bq, Wk, bk, Wv, bv):
    import ml_dtypes

    bf16 = ml_dtypes.bfloat16
    fp8 = ml_dtypes.float8_e4m3fn
    q = np.asarray(q, np.float32)
    k = np.asarray(k, np.float32)
    v = np.asarray(v, np.float32)
    mask = np.asarray(mask, np.int32)
    Wq = np.asarray(Wq, np.float32)
    Wk = np.asarray(Wk, np.float32)
    Wv = np.asarray(Wv, np.float32)

    use_bias = bool(
        np.any(np.asarray(bq, np.float32))
        or np.any(np.asarray(bk, np.float32))
        or np.any(np.asarray(bv, np.float32))
    )

    idxs = [np.nonzero(mask[b])[0] for b in range(B)]
    neff = max(1, max(len(ix) for ix in idxs))
    nk = -(-neff // P) * P  # round up to multiple of 128

    def pair4(x):  # [D, w] -> [P, ND2, 2, w] fp8 (d = d2*256 + ko*128 + p)
        w = x.shape[1]
        return np.ascontiguousarray(
            x.reshape(ND2, 2, P, w).transpose(2, 0, 1, 3)
        ).astype(fp8)

    def tile8(x):  # [D, w] -> [P, ND, w]
        w = x.shape[1]
        return np.ascontiguousarray(x.reshape(ND, P, w).transpose(1, 0, 2))

    qxs, kxs, vxs, mks = [], [], [], []
    for b in range(B):
        ix = idxs[b]
        qxs.append(pair4(q[b].T))
        kc = np.zeros((D, nk), np.float32)
        vc = np.zeros((D, nk), np.float32)
        kc[:, :len(ix)] = k[b].T[:, ix]
        vc[:, :len(ix)] = v[b].T[:, ix]
        kxs.append(pair4(kc))
        vxs.append(tile8(vc).astype(bf16))
        m = np.zeros((nk,), np.int32)
        m[:len(ix)] = 1
        mks.append(m)

    in_maps = []
    for c in range(NCORES):
        b, g = divmod(c, GROUPS)
        cols = slice(g * GW, (g + 1) * GW)
        im = {
            "qx": qxs[b],
            "kx": kxs[b],
            "vx": vxs[b],
            "wq": pair4(Wq[:, cols]),
            "wk": pair4(Wk[:, cols]),
            "wv": tile8(Wv[:, cols]).astype(bf16),
            "mask": mks[b],
        }
        if use_bias:
            im["bq"] = np.ascontiguousarray(bq[cols]).astype(bf16)
            im["bk"] = np.ascontiguousarray(bk[cols]).astype(bf16)
            im["bv"] = np.ascontiguousarray(bv[cols]).astype(bf16)
        in_maps.append(im)
    return nk, use_bias, in_maps


def _assemble(results):
    out = np.empty((B, S, D), np.float32)
    for c in range(NCORES):
        b, g = divmod(c, GROUPS)
        out[b, :, g * GW:(g + 1) * GW] = results[c]["out"].astype(np.float32)
    return out


def kernel(q, k, v, mask, Wq, bq, Wk, bk, Wv, bv):
    nk, use_bias, in_maps = _make_in_maps(q, k, v, mask, Wq, bq, Wk, bk, Wv, bv)
    res = _run(_get_nc(nk, use_bias), in_maps, trace=False)
    return _assemble(res.results)


def _install_ntff_hook():
    """The image's antenv stub lacks axon_hooks; synthesize it and register
    the ctypes NTFF hook that trn_agent_boot would have installed."""
    import sys
    import types

    import antenv

    if "antenv.axon_hooks" in sys.modules:
        return
    mod = types.ModuleType("antenv.axon_hooks")
    state = {"hook": None}
    mod.set_axon_ntff_profile_hook = lambda h: state.__setitem__("hook", h)
    mod.get_axon_ntff_profile_hook = lambda: state["hook"]
    sys.modules["antenv.axon_hooks"] = mod
    antenv.axon_hooks = mod
    try:
        from trn_agent_boot.trn_boot import _ntff_profile_via_ctypes

        mod.set_axon_ntff_profile_hook(
            _ntff_profile_via_ctypes("/opt/axon/libaxon_pjrt.so")
        )
    except Exception as e:
        print(f"ntff hook registration failed: {e}")


def _exec_ns_from_newest_ntff():
    """Span of the newest NTFF json's DMA events — matches gauge's
    first/last-useful exec time when instruction events are absent."""
    import glob
    import json as _json
    import os

    try:
        path = max(glob.glob("/tmp/tmp*/ntff_0.json"), key=os.path.getmtime)
        d = _json.load(open(path))
        ev = d.get("dma", [])
        if not ev:
            return None
        t0 = min(e["timestamp"] for e in ev)
        t1 = max(e["timestamp"] + e.get("duration", 0) for e in ev)
        return t1 - t0
    except Exception:
        return None


def kernel_traced(q, k, v, mask, Wq, bq, Wk, bk, Wv, bv):
    """Same as kernel() but also returns (output, exec_time_ns)."""
    _install_ntff_hook()
    nk, use_bias, in_maps = _make_in_maps(q, k, v, mask, Wq, bq, Wk, bk, Wv, bv)
    nc = _get_nc(nk, use_bias)
    try:
        res = _run(nc, in_maps, trace=True)
        return _assemble(res.results), res.exec_time_ns
    except Exception:
        # gauge's NTFF->perfetto step can fail on kernels whose profile
        # lacks instruction events (`assert insts`); the NTFF json still
        # exists, so recover the exec time from its DMA span and rerun
        # untraced for the outputs.
        exec_ns = _exec_ns_from_newest_ntff()
        res = _run(nc, in_maps, trace=False)
        return _assemble(res.results), exec_ns


# revision 15
# speedup vs baseline: 1.1599x; 1.0058x over previous
"""Bass/Trainium2 kernel for nn_Attention_28140625723842 (v2).

Multi-head attention (B=2, S=2048, D=1024, H=16, DH=64) with key-padding
mask, sharded over 8 NeuronCores as 2 batches x 4 head-groups.

v2 design (vs the v1 baseline at ~150-162us):
  - Q/K projections run in fp8(e4m3) DoubleRow perf mode: contraction
    pairs (d, d+128) packed per PE cell -> ~1.8x matmul throughput and
    half the activation DMA bytes. V projection stays bf16 (its error
    feeds the output directly).
  - k/v tokens host-compacted to the unmasked set (padded to 128).
  - Attention runs as 4 blocks (hp head-pair x qb half-of-S). Per k-tile
    the two heads of the pair are two phase-offset streams (psA/psB,
    each single-buffered): while ScalarE exps stream A then B, the PE
    refills the other bank -> ScalarE (the wall at ~75us) stays
    saturated.
  - P@V for block n is woven into block n+1's k-tile loop (PSUM has
    exactly 8 banks: 2+2 for scores + 4 for the deferred accumulators).
    Block 0's weave slots instead run the V projection.
  - The V matrix carries a ones column, so the softmax denominator falls
    out of the P@V matmul as row DH.
  - Normalize tail without PE transposes: PV accumulator -> bf16 SBUF
    copy -> DMA-engine transpose [96,1024]->[128,8,96] -> one batched
    DVE reciprocal + one broadcasted tensor_tensor multiply per head.
  - Output bf16, DMA'd per block as soon as it is normalized; host
    upcasts to fp32.
"""

import numpy as np

B, S, D, H = 2, 2048, 1024, 16
DH = D // H            # 64 head dim
NCORES = 8
GROUPS = NCORES // B   # 4 head groups
HL = H // GROUPS       # 4 heads per core
GW = HL * DH           # 256 output columns per core

P = 128
ND = D // P            # 8 contraction tiles (bf16 path)
ND2 = D // 256         # 4 DoubleRow contraction tiles (fp8 path)
NT = S // P            # 16 q token tiles
QB = 1024              # q block (one exp op width)
NQB = S // QB          # 2
NQ8 = QB // P          # 8 q token tiles per block
CH = 512               # matmul free-dim chunk (one PSUM bank fp32)
NCH = QB // CH         # 2

_CACHE = {}


def _chunks(total, width):
    out = []
    o = 0
    while o < total:
        w = min(width, total - o)
        out.append((o, w))
        o += w
    return out


def _build_nc(nk, use_bias=False):
    import concourse.bacc as bacc
    import concourse.mybir as mybir
    import concourse.tile as tile
    from concourse.bass import broadcast_tensor_aps

    f32 = mybir.dt.float32
    bf16 = mybir.dt.bfloat16
    fp8 = mybir.dt.float8e4
    i32 = mybir.dt.int32
    Exp = mybir.ActivationFunctionType.Exp
    DR = mybir.MatmulPerfMode.DoubleRow
    SCALE = float(1.0 / np.sqrt(np.float32(D)))
    NTK = nk // P          # k token tiles (compacted)

    nc = bacc.Bacc(None, target_bir_lowering=False)
    qx_d = nc.dram_tensor("qx", [P, ND2, 2, S], fp8, kind="ExternalInput")
    kx_d = nc.dram_tensor("kx", [P, ND2, 2, nk], fp8, kind="ExternalInput")
    vx_d = nc.dram_tensor("vx", [P, ND, nk], bf16, kind="ExternalInput")
    wq_d = nc.dram_tensor("wq", [P, ND2, 2, GW], fp8, kind="ExternalInput")
    wk_d = nc.dram_tensor("wk", [P, ND2, 2, GW], fp8, kind="ExternalInput")
    wv_d = nc.dram_tensor("wv", [P, ND, GW], bf16, kind="ExternalInput")
    mask_d = nc.dram_tensor("mask", [nk], i32, kind="ExternalInput")
    out_d = nc.dram_tensor("out", [S, GW], bf16, kind="ExternalOutput")
    if use_bias:
        bq_d = nc.dram_tensor("bq", [GW], bf16, kind="ExternalInput")
        bk_d = nc.dram_tensor("bk", [GW], bf16, kind="ExternalInput")
        bv_d = nc.dram_tensor("bv", [GW], bf16, kind="ExternalInput")

    with tile.TileContext(nc) as tc:
        with (
            tc.tile_pool(name="consts", bufs=1) as consts,
            tc.tile_pool(name="persist", bufs=1) as persist,
            tc.tile_pool(name="exps", bufs=24) as expp,
            tc.tile_pool(name="pvsb", bufs=4) as pvsbp,
            tc.tile_pool(name="tpsb", bufs=4) as tpsbp,
            tc.tile_pool(name="recs", bufs=4) as recsp,
        ):
            # mask[k] -> per-partition exp bias: (m - 1) * 1e9  (0 or -1e9)
            maski = consts.tile([P, NTK], i32, tag="maski")
            nc.scalar.dma_start(maski, mask_d.rearrange("(t p) -> p t", p=P))
            maskb = consts.tile([P, NTK], f32, tag="maskb")
            nc.vector.tensor_scalar(
                maskb, maski, -1.0, 1e9,
                mybir.AluOpType.add, mybir.AluOpType.mult,
            )
            # tiny dummy exp to pull the ~1.3us ACT_TABLE_LOAD off the
            # critical path (runs during the input-DMA ramp)
            warm = consts.tile([1, 1], f32, tag="warm")
            nc.scalar.activation(warm, maskb[0:1, 0:1], Exp)

            brow = {}
            if use_bias:
                ones = consts.tile([1, CH], bf16, tag="ones")
                nc.vector.memset(ones, 1.0)
                for nm, drm in (("q", bq_d), ("k", bk_d), ("v", bv_d)):
                    t = consts.tile([1, GW], bf16, tag=f"bias_{nm}")
                    nc.scalar.dma_start(t, drm[None, :])
                    brow[nm] = t

            # weights first (small, unblock first matmuls)
            wk_sb = persist.tile([P, ND2, 2, GW], fp8, tag="wk")
            nc.scalar.dma_start(wk_sb, wk_d[:, :, :, :])
            wq_sb = persist.tile([P, ND2, 2, GW], fp8, tag="wq")
            nc.scalar.dma_start(wq_sb, wq_d[:, :, :, :])
            wv_sb = persist.tile([P, ND, GW], bf16, tag="wv")
            nc.scalar.dma_start(wv_sb, wv_d[:, :, :])

            # k activations on the sync ring, chunked by (dt2, nk-half)
            kx_sb = persist.tile([P, ND2, 2, nk], fp8, tag="kx")
            nkh = (NTK // 2) * P
            for d2 in range(ND2):
                nc.sync.dma_start(kx_sb[:, d2, :, :nkh],
                                  kx_d[:, d2, :, :nkh])
            for d2 in range(ND2):
                nc.sync.dma_start(kx_sb[:, d2, :, nkh:],
                                  kx_d[:, d2, :, nkh:])
            qx_sb = persist.tile([P, ND2, 2, S], fp8, tag="qx")
            for qb in range(NQB):
                qs = slice(qb * QB, (qb + 1) * QB)
                for d2 in range(ND2):
                    nc.sync.dma_start(qx_sb[:, d2, :, qs], qx_d[:, d2, :, qs])
            # v activations on the scalar ring (parallel to qx)
            vx_sb = persist.tile([P, ND, nk], bf16, tag="vx")
            nc.scalar.dma_start(vx_sb, vx_d[:, :, :])

            QT = persist.tile([P, 2, S], bf16, tag="QT")
            KT = persist.tile([P, 2, nk], bf16, tag="KT")
            V = persist.tile([P, NTK, HL * (DH + 1)], bf16, tag="V")
            V4 = V.rearrange("p t (h e) -> p t h e", h=HL)
            out_sb = persist.tile([P, NT, GW], bf16, tag="osb")
            out_blk = out_d.rearrange("(t p) w -> p t w", p=P)

            for h in range(HL):
                nc.vector.memset(V4[:, :, h, DH], 1.0)

            with (
                tc.tile_pool(name="pssA", bufs=1, space="PSUM") as pssA,
                tc.tile_pool(name="pssB", bufs=1, space="PSUM") as pssB,
            ):
                def proj_qk_group(which, x_sb, w_sb, OUT, hp, co, cw):
                    # one fp8 DoubleRow projection group: OUT[:, hp, co:co+cw]
                    pt = pp.tile([P, CH], f32, tag="pp")
                    for d2 in range(ND2):
                        nc.tensor.matmul(
                            pt[:, :cw],
                            lhsT=w_sb[:, d2, :, hp * P:(hp + 1) * P],
                            rhs=x_sb[:, d2, :, co:co + cw],
                            start=(d2 == 0),
                            stop=(not use_bias and d2 == ND2 - 1),
                            perf_mode=DR,
                        )
                    if use_bias:
                        nc.tensor.matmul(
                            pt[:, :cw],
                            lhsT=brow[which][:, hp * P:(hp + 1) * P],
                            rhs=ones[:, :cw],
                            start=False, stop=True,
                        )
                    nc.vector.tensor_copy(
                        out=OUT[:, hp, co:co + cw], in_=pt[:, :cw]
                    )

                def emit_vproj(tt):
                    # one V-projection token tile (bf16), woven into block 0
                    vp = pp.tile([P, CH], f32, tag="pp")
                    for dt_ in range(ND):
                        nc.tensor.matmul(
                            vp[:, :GW],
                            lhsT=vx_sb[:, dt_, tt * P:(tt + 1) * P],
                            rhs=wv_sb[:, dt_, :],
                            start=(dt_ == 0),
                            stop=(not use_bias and dt_ == ND - 1),
                        )
                    if use_bias:
                        nc.tensor.matmul(
                            vp[:, :GW],
                            lhsT=ones[:, :P],
                            rhs=brow["v"],
                            start=False, stop=True,
                        )
                    nc.vector.tensor_copy(
                        out=V4[:, tt, :, :DH],
                        in_=vp[:, :GW].rearrange("p (h e) -> p h e", h=HL),
                    )

                def emit_block(hp, qb, weave_pv, weave_v, extra_weave=None):
                    # one attention block: two phase-offset exp streams.
                    # weave_pv: (pvts, ets, ...) of the PREVIOUS block whose
                    # P@V matmuls ride this block's k-tile loop.
                    # Weave work is emitted BEFORE the unit's scores so a
                    # stalled score matmul (WAR on its exp) never head-of-line
                    # blocks the weave, and the weave never delays the next
                    # score dispatch past the exp window.
                    ets = ([], [])
                    for kt in range(NTK):
                        if weave_pv is not None:
                            emit_pv_unit(weave_pv, kt)
                        if weave_v:
                            emit_vproj(kt)
                        if extra_weave and kt >= 4:
                            extra_weave.pop(0)()
                        for st, pool in ((0, pssA), (1, pssB)):
                            ps = pool.tile([P, QB], f32, tag=f"s{st}")
                            po = st * DH
                            for c in range(NCH):
                                nc.tensor.matmul(
                                    ps[:, c * CH:(c + 1) * CH],
                                    lhsT=KT[po:po + DH, hp,
                                            kt * P:(kt + 1) * P],
                                    rhs=QT[po:po + DH, hp,
                                           qb * QB + c * CH:
                                           qb * QB + (c + 1) * CH],
                                    start=True, stop=True,
                                )
                            e = expp.tile([P, QB], bf16, tag="e")
                            nc.scalar.activation(
                                e, ps, Exp,
                                bias=maskb[:, kt:kt + 1], scale=SCALE,
                            )
                            ets[st].append(e)
                    return ets

                def emit_pv_unit(pv_state, kt):
                    pvts, p_ets, _, _ = pv_state
                    for st in range(2):
                        for c in range(NCH):
                            nc.tensor.matmul(
                                pvts[st][:, c * CH:(c + 1) * CH],
                                lhsT=V[:, kt,
                                       (2 * pv_state[2] + st) * (DH + 1):
                                       (2 * pv_state[2] + st + 1) * (DH + 1)],
                                rhs=p_ets[st][kt][:, c * CH:(c + 1) * CH],
                                start=(kt == 0), stop=(kt == NTK - 1),
                            )

                def emit_normalize(pv_state):
                    # prev block's accumulators -> bf16 -> DMA transpose ->
                    # batched reciprocal -> broadcast multiply -> out DMA
                    pvts, _, hp_p, qb_p = pv_state
                    for st in range(2):
                        pv_sb = pvsbp.tile([96, QB], bf16, tag="pvsb")
                        nc.vector.tensor_copy(
                            out=pv_sb[:DH + 1, :], in_=pvts[st]
                        )
                        tps = tpsbp.tile([P, NQ8, 96], bf16, tag="tps")
                        nc.sync.dma_start_transpose(tps, pv_sb[:, :])
                        rec = recsp.tile([P, NQ8, 1], f32, tag="rec")
                        nc.vector.reciprocal(rec, tps[:, :, DH:DH + 1])
                        col = hp_p * P + st * DH
                        o_ap = out_sb[:, qb_p * NQ8:(qb_p + 1) * NQ8,
                                      col:col + DH]
                        in0 = tps[:, :, :DH]
                        in0b, in1b = broadcast_tensor_aps(in0, rec)
                        nc.vector.tensor_tensor(
                            out=o_ap, in0=in0b, in1=in1b,
                            op=mybir.AluOpType.mult,
                        )
                    nc.sync.dma_start(
                        out_blk[:, qb_p * NQ8:(qb_p + 1) * NQ8,
                                hp_p * P:(hp_p + 1) * P],
                        out_sb[:, qb_p * NQ8:(qb_p + 1) * NQ8,
                               hp_p * P:(hp_p + 1) * P],
                    )

                blocks = [(0, 0), (0, 1), (1, 0), (1, 1)]

                with tc.tile_pool(name="pp", bufs=2, space="PSUM") as pp:
                    # upfront projections (fp8 DoubleRow): all of K (its DMA
                    # lands first), then the qb=0 half of Q
                    for hp in range(2):
                        for co, cw in _chunks(nk, CH):
                            proj_qk_group("k", kx_sb, wk_sb, KT, hp, co, cw)
                    for hp in range(2):
                        for c in range(NCH):
                            proj_qk_group("q", qx_sb, wq_sb, QT,
                                          hp, c * CH, CH)
                    # qb=1 Q-projection groups ride block 0's later units
                    # (their qx DMA lands last); V projection rides every unit
                    qb1 = [
                        (lambda hp=hp, c=c: proj_qk_group(
                            "q", qx_sb, wq_sb, QT, hp, QB + c * CH, CH))
                        for hp in range(2) for c in range(NCH)
                    ]
                    ets0 = emit_block(*blocks[0], weave_pv=None, weave_v=True,
                                      extra_weave=qb1)

                with tc.tile_pool(name="pvt", bufs=2, space="PSUM") as pvtp:
                    prev = None
                    prev_ets = ets0
                    prev_blk = blocks[0]
                    for bi in range(1, 4):
                        hp, qb = blocks[bi]
                        pvts = [
                            pvtp.tile([DH + 1, QB], f32, tag="pvt",
                                      name=f"pvt_{bi}_{st}")
                            for st in range(2)
                        ]
                        pv_state = (pvts, prev_ets, prev_blk[0], prev_blk[1])
                        ets = emit_block(hp, qb, weave_pv=pv_state,
                                         weave_v=False)
                        emit_normalize(pv_state)
                        prev_ets = ets
                        prev_blk = blocks[bi]
                    # tail: last block's P@V + normalize
                    pvts = [
                        pvtp.tile([DH + 1, QB], f32, tag="pvt",
                                  name=f"pvt_tail_{st}")
                        for st in range(2)
                    ]
                    pv_state = (pvts, prev_ets, prev_blk[0], prev_blk[1])
                    for kt in range(NTK):
                        emit_pv_unit(pv_state, kt)
                    emit_normalize(pv_state)
    nc.compile()
    return nc


def _get_nc(nk, use_bias=False):
    key = (nk, use_bias)
    if key not in _CACHE:
        _CACHE[key] = _build_nc(nk, use_bias=use_bias)
    return _CACHE[key]


def _run(nc, in_maps, trace=False):
    from concourse.bass_utils import run_bass_kernel_spmd

    return run_bass_kernel_spmd(
        nc, in_maps, core_ids=list(range(NCORES)), trace=trace
    )


def _make_in_maps(q, k, v, mask, Wq, bq, Wk, bk, Wv, bv):
    import ml_dtypes

    bf16 = ml_dtypes.bfloat16
    fp8 = ml_dtypes.float8_e4m3fn
    q = np.asarray(q, np.float32)
    k = np.asarray(k, np.float32)
    v = np.asarray(v, np.float32)
    mask = np.asarray(mask, np.int32)
    Wq = np.asarray(Wq, np.float32)
    Wk = np.asarray(Wk, np.float32)
    Wv = np.asarray(Wv, np.float32)

    use_bias = bool(
        np.any(np.asarray(bq, np.float32))
        or np.any(np.asarray(bk, np.float32))
        or np.any(np.asarray(bv, np.float32))
    )

    idxs = [np.nonzero(mask[b])[0] for b in range(B)]
    neff = max(1, max(len(ix) for ix in idxs))
    nk = -(-neff // P) * P  # round up to multiple of 128

    def pair4(x):  # [D, w] -> [P, ND2, 2, w] fp8 (d = d2*256 + ko*128 + p)
        w = x.shape[1]
        return np.ascontiguousarray(
            x.reshape(ND2, 2, P, w).transpose(2, 0, 1, 3)
        ).astype(fp8)

    def tile8(x):  # [D, w] -> [P, ND, w]
        w = x.shape[1]
        return np.ascontiguousarray(x.reshape(ND, P, w).transpose(1, 0, 2))

    qxs, kxs, vxs, mks = [], [], [], []
    for b in range(B):
        ix = idxs[b]
        qxs.append(pair4(q[b].T))
        kc = np.zeros((D, nk), np.float32)
        vc = np.zeros((D, nk), np.float32)
        kc[:, :len(ix)] = k[b].T[:, ix]
        vc[:, :len(ix)] = v[b].T[:, ix]
        kxs.append(pair4(kc))
        vxs.append(tile8(vc).astype(bf16))
        m = np.zeros((nk,), np.int32)
        m[:len(ix)] = 1
        mks.append(m)

    in_maps = []
    for c in range(NCORES):
        b, g = divmod(c, GROUPS)
        cols = slice(g * GW, (g + 1) * GW)
        im = {
            "qx": qxs[b],
            "kx": kxs[b],
            "vx": vxs[b],
            "wq": pair4(Wq[:, cols]),
            "wk": pair4(Wk[:, cols]),
            "wv": tile8(Wv[:, cols]).astype(bf16),
            "mask": mks[b],
        }
        if use_bias:
            im["bq"] = np.ascontiguousarray(bq[cols]).astype(bf16)
            im["bk"] = np.ascontiguousarray(bk[cols]).astype(bf16)
            im["bv"] = np.ascontiguousarray(bv[cols]).astype(bf16)
        in_maps.append(im)
    return nk, use_bias, in_maps


def _assemble(results):
    out = np.empty((B, S, D), np.float32)
    for c in range(NCORES):
        b, g = divmod(c, GROUPS)
        out[b, :, g * GW:(g + 1) * GW] = results[c]["out"].astype(np.float32)
    return out


def kernel(q, k, v, mask, Wq, bq, Wk, bk, Wv, bv):
    nk, use_bias, in_maps = _make_in_maps(q, k, v, mask, Wq, bq, Wk, bk, Wv, bv)
    res = _run(_get_nc(nk, use_bias), in_maps, trace=False)
    return _assemble(res.results)


def _install_ntff_hook():
    """The image's antenv stub lacks axon_hooks; synthesize it and register
    the ctypes NTFF hook that trn_agent_boot would have installed."""
    import sys
    import types

    import antenv

    if "antenv.axon_hooks" in sys.modules:
        return
    mod = types.ModuleType("antenv.axon_hooks")
    state = {"hook": None}
    mod.set_axon_ntff_profile_hook = lambda h: state.__setitem__("hook", h)
    mod.get_axon_ntff_profile_hook = lambda: state["hook"]
    sys.modules["antenv.axon_hooks"] = mod
    antenv.axon_hooks = mod
    try:
        from trn_agent_boot.trn_boot import _ntff_profile_via_ctypes

        mod.set_axon_ntff_profile_hook(
            _ntff_profile_via_ctypes("/opt/axon/libaxon_pjrt.so")
        )
    except Exception as e:
        print(f"ntff hook registration failed: {e}")


def _exec_ns_from_newest_ntff():
    """Span of the newest NTFF json's DMA events — matches gauge's
    first/last-useful exec time when instruction events are absent."""
    import glob
    import json as _json
    import os

    try:
        path = max(glob.glob("/tmp/tmp*/ntff_0.json"), key=os.path.getmtime)
        d = _json.load(open(path))
        ev = d.get("dma", [])
        if not ev:
            return None
        t0 = min(e["timestamp"] for e in ev)
        t1 = max(e["timestamp"] + e.get("duration", 0) for e in ev)
        return t1 - t0
    except Exception:
        return None


def kernel_traced(q, k, v, mask, Wq, bq, Wk, bk, Wv, bv):
    """Same as kernel() but also returns (output, exec_time_ns)."""
    _install_ntff_hook()
    nk, use_bias, in_maps = _make_in_maps(q, k, v, mask, Wq, bq, Wk, bk, Wv, bv)
    nc = _get_nc(nk, use_bias)
    try:
        res = _run(nc, in_maps, trace=True)
        return _assemble(res.results), res.exec_time_ns
    except Exception:
        # gauge's NTFF->perfetto step can fail on kernels whose profile
        # lacks instruction events (`assert insts`); the NTFF json still
        # exists, so recover the exec time from its DMA span and rerun
        # untraced for the outputs.
        exec_ns = _exec_ns_from_newest_ntff()
        res = _run(nc, in_maps, trace=False)
        return _assemble(res.results), exec_ns


# revision 17
# speedup vs baseline: 1.1899x; 1.0259x over previous
"""Bass/Trainium2 kernel for nn_Attention_28140625723842 (v2).

Multi-head attention (B=2, S=2048, D=1024, H=16, DH=64) with key-padding
mask, sharded over 8 NeuronCores as 2 batches x 4 head-groups.

v2 design (vs the v1 baseline at ~150-162us):
  - Q/K projections run in fp8(e4m3) DoubleRow perf mode: contraction
    pairs (d, d+128) packed per PE cell -> ~1.8x matmul throughput and
    half the activation DMA bytes. V projection stays bf16 (its error
    feeds the output directly).
  - k/v tokens host-compacted to the unmasked set (padded to 128).
  - Attention runs as 4 blocks (hp head-pair x qb half-of-S). Per k-tile
    the two heads of the pair are two phase-offset streams (psA/psB,
    each single-buffered): while ScalarE exps stream A then B, the PE
    refills the other bank -> ScalarE (the wall at ~75us) stays
    saturated.
  - P@V for block n is woven into block n+1's k-tile loop (PSUM has
    exactly 8 banks: 2+2 for scores + 4 for the deferred accumulators).
    Block 0's weave slots instead run the V projection.
  - The V matrix carries a ones column, so the softmax denominator falls
    out of the P@V matmul as row DH.
  - Normalize tail without PE transposes: PV accumulator -> bf16 SBUF
    copy -> DMA-engine transpose [96,1024]->[128,8,96] -> one batched
    DVE reciprocal + one broadcasted tensor_tensor multiply per head.
  - Output bf16, DMA'd per block as soon as it is normalized; host
    upcasts to fp32.
"""

import numpy as np

B, S, D, H = 2, 2048, 1024, 16
DH = D // H            # 64 head dim
NCORES = 8
GROUPS = NCORES // B   # 4 head groups
HL = H // GROUPS       # 4 heads per core
GW = HL * DH           # 256 output columns per core

P = 128
ND = D // P            # 8 contraction tiles (bf16 path)
ND2 = D // 256         # 4 DoubleRow contraction tiles (fp8 path)
NT = S // P            # 16 q token tiles
QB = 1024              # q block (one exp op width)
NQB = S // QB          # 2
NQ8 = QB // P          # 8 q token tiles per block
CH = 512               # matmul free-dim chunk (one PSUM bank fp32)
NCH = QB // CH         # 2

_CACHE = {}


def _chunks(total, width):
    out = []
    o = 0
    while o < total:
        w = min(width, total - o)
        out.append((o, w))
        o += w
    return out


def _build_nc(nk, use_bias=False):
    import concourse.bacc as bacc
    import concourse.mybir as mybir
    import concourse.tile as tile
    from concourse.bass import broadcast_tensor_aps

    f32 = mybir.dt.float32
    bf16 = mybir.dt.bfloat16
    fp8 = mybir.dt.float8e4
    i32 = mybir.dt.int32
    Exp = mybir.ActivationFunctionType.Exp
    DR = mybir.MatmulPerfMode.DoubleRow
    SCALE = float(1.0 / np.sqrt(np.float32(D)))
    NTK = nk // P          # k token tiles (compacted)

    nc = bacc.Bacc(None, target_bir_lowering=False)
    qx_d = nc.dram_tensor("qx", [P, ND2, 2, S], fp8, kind="ExternalInput")
    kx_d = nc.dram_tensor("kx", [P, ND2, 2, nk], fp8, kind="ExternalInput")
    vx_d = nc.dram_tensor("vx", [P, ND, nk], bf16, kind="ExternalInput")
    wq_d = nc.dram_tensor("wq", [P, ND2, 2, GW], fp8, kind="ExternalInput")
    wk_d = nc.dram_tensor("wk", [P, ND2, 2, GW], fp8, kind="ExternalInput")
    wv_d = nc.dram_tensor("wv", [P, ND, GW], bf16, kind="ExternalInput")
    mask_d = nc.dram_tensor("mask", [nk], i32, kind="ExternalInput")
    out_d = nc.dram_tensor("out", [S, GW], bf16, kind="ExternalOutput")
    if use_bias:
        bq_d = nc.dram_tensor("bq", [GW], bf16, kind="ExternalInput")
        bk_d = nc.dram_tensor("bk", [GW], bf16, kind="ExternalInput")
        bv_d = nc.dram_tensor("bv", [GW], bf16, kind="ExternalInput")

    with tile.TileContext(nc) as tc:
        with (
            tc.tile_pool(name="consts", bufs=1) as consts,
            tc.tile_pool(name="persist", bufs=1) as persist,
            tc.tile_pool(name="exps", bufs=24) as expp,
            tc.tile_pool(name="pvsb", bufs=4) as pvsbp,
            tc.tile_pool(name="tpsb", bufs=4) as tpsbp,
            tc.tile_pool(name="recs", bufs=4) as recsp,
        ):
            # mask[k] -> per-partition exp bias: (m - 1) * 1e9  (0 or -1e9)
            maski = consts.tile([P, NTK], i32, tag="maski")
            nc.scalar.dma_start(maski, mask_d.rearrange("(t p) -> p t", p=P))
            maskb = consts.tile([P, NTK], f32, tag="maskb")
            nc.vector.tensor_scalar(
                maskb, maski, -1.0, 1e9,
                mybir.AluOpType.add, mybir.AluOpType.mult,
            )


            brow = {}
            if use_bias:
                ones = consts.tile([1, CH], bf16, tag="ones")
                nc.vector.memset(ones, 1.0)
                for nm, drm in (("q", bq_d), ("k", bk_d), ("v", bv_d)):
                    t = consts.tile([1, GW], bf16, tag=f"bias_{nm}")
                    nc.scalar.dma_start(t, drm[None, :])
                    brow[nm] = t

            # weights first (small, unblock first matmuls)
            wk_sb = persist.tile([P, ND2, 2, GW], fp8, tag="wk")
            nc.scalar.dma_start(wk_sb, wk_d[:, :, :, :])
            wq_sb = persist.tile([P, ND2, 2, GW], fp8, tag="wq")
            nc.scalar.dma_start(wq_sb, wq_d[:, :, :, :])
            wv_sb = persist.tile([P, ND, GW], bf16, tag="wv")
            nc.scalar.dma_start(wv_sb, wv_d[:, :, :])

            # k activations on the sync ring, chunked by dt2
            kx_sb = persist.tile([P, ND2, 2, nk], fp8, tag="kx")
            for d2 in range(ND2):
                nc.sync.dma_start(kx_sb[:, d2], kx_d[:, d2, :, :])
            qx_sb = persist.tile([P, ND2, 2, S], fp8, tag="qx")
            for qb in range(NQB):
                qs = slice(qb * QB, (qb + 1) * QB)
                for d2 in range(ND2):
                    nc.sync.dma_start(qx_sb[:, d2, :, qs], qx_d[:, d2, :, qs])
            # v activations on the scalar ring (parallel to qx)
            vx_sb = persist.tile([P, ND, nk], bf16, tag="vx")
            nc.scalar.dma_start(vx_sb, vx_d[:, :, :])

            QT = persist.tile([P, 2, S], bf16, tag="QT")
            KT = persist.tile([P, 2, nk], bf16, tag="KT")
            V = persist.tile([P, NTK, HL * (DH + 1)], bf16, tag="V")
            V4 = V.rearrange("p t (h e) -> p t h e", h=HL)
            out_sb = persist.tile([P, NT, GW], bf16, tag="osb")
            out_blk = out_d.rearrange("(t p) w -> p t w", p=P)

            for h in range(HL):
                nc.vector.memset(V4[:, :, h, DH], 1.0)

            with (
                tc.tile_pool(name="pssA", bufs=1, space="PSUM") as pssA,
                tc.tile_pool(name="pssB", bufs=1, space="PSUM") as pssB,
            ):
                def proj_qk_group(which, x_sb, w_sb, OUT, hp, co, cw):
                    # one fp8 DoubleRow projection group: OUT[:, hp, co:co+cw]
                    pt = pp.tile([P, CH], f32, tag="pp")
                    for d2 in range(ND2):
                        nc.tensor.matmul(
                            pt[:, :cw],
                            lhsT=w_sb[:, d2, :, hp * P:(hp + 1) * P],
                            rhs=x_sb[:, d2, :, co:co + cw],
                            start=(d2 == 0),
                            stop=(not use_bias and d2 == ND2 - 1),
                            perf_mode=DR,
                        )
                    if use_bias:
                        nc.tensor.matmul(
                            pt[:, :cw],
                            lhsT=brow[which][:, hp * P:(hp + 1) * P],
                            rhs=ones[:, :cw],
                            start=False, stop=True,
                        )
                    nc.vector.tensor_copy(
                        out=OUT[:, hp, co:co + cw], in_=pt[:, :cw]
                    )

                def emit_vproj(tt):
                    # one V-projection token tile (bf16), woven into block 0
                    vp = pp.tile([P, CH], f32, tag="pp")
                    for dt_ in range(ND):
                        nc.tensor.matmul(
                            vp[:, :GW],
                            lhsT=vx_sb[:, dt_, tt * P:(tt + 1) * P],
                            rhs=wv_sb[:, dt_, :],
                            start=(dt_ == 0),
                            stop=(not use_bias and dt_ == ND - 1),
                        )
                    if use_bias:
                        nc.tensor.matmul(
                            vp[:, :GW],
                            lhsT=ones[:, :P],
                            rhs=brow["v"],
                            start=False, stop=True,
                        )
                    nc.vector.tensor_copy(
                        out=V4[:, tt, :, :DH],
                        in_=vp[:, :GW].rearrange("p (h e) -> p h e", h=HL),
                    )

                def emit_block(hp, qb, weave_pv, weave_v, extra_weave=None):
                    # one attention block: two phase-offset exp streams.
                    # weave_pv: (pvts, ets, ...) of the PREVIOUS block whose
                    # P@V matmuls ride this block's k-tile loop.
                    # Weave work is emitted BEFORE the unit's scores so a
                    # stalled score matmul (WAR on its exp) never head-of-line
                    # blocks the weave, and the weave never delays the next
                    # score dispatch past the exp window.
                    ets = ([], [])
                    for kt in range(NTK):
                        if weave_pv is not None:
                            emit_pv_unit(weave_pv, kt)
                        if weave_v:
                            emit_vproj(kt)
                        if extra_weave and kt >= 4:
                            extra_weave.pop(0)()
                        for st, pool in ((0, pssA), (1, pssB)):
                            ps = pool.tile([P, QB], f32, tag=f"s{st}")
                            po = st * DH
                            for c in range(NCH):
                                nc.tensor.matmul(
                                    ps[:, c * CH:(c + 1) * CH],
                                    lhsT=KT[po:po + DH, hp,
                                            kt * P:(kt + 1) * P],
                                    rhs=QT[po:po + DH, hp,
                                           qb * QB + c * CH:
                                           qb * QB + (c + 1) * CH],
                                    start=True, stop=True,
                                )
                            e = expp.tile([P, QB], bf16, tag="e")
                            nc.scalar.activation(
                                e, ps, Exp,
                                bias=maskb[:, kt:kt + 1], scale=SCALE,
                            )
                            ets[st].append(e)
                    return ets

                def emit_pv_unit(pv_state, kt):
                    pvts, p_ets, _, _ = pv_state
                    for st in range(2):
                        for c in range(NCH):
                            nc.tensor.matmul(
                                pvts[st][:, c * CH:(c + 1) * CH],
                                lhsT=V[:, kt,
                                       (2 * pv_state[2] + st) * (DH + 1):
                                       (2 * pv_state[2] + st + 1) * (DH + 1)],
                                rhs=p_ets[st][kt][:, c * CH:(c + 1) * CH],
                                start=(kt == 0), stop=(kt == NTK - 1),
                            )

                def emit_normalize(pv_state):
                    # prev block's accumulators -> bf16 -> DMA transpose ->
                    # batched reciprocal -> broadcast multiply -> out DMA
                    pvts, _, hp_p, qb_p = pv_state
                    for st in range(2):
                        pv_sb = pvsbp.tile([96, QB], bf16, tag="pvsb")
                        nc.vector.tensor_copy(
                            out=pv_sb[:DH + 1, :], in_=pvts[st]
                        )
                        tps = tpsbp.tile([P, NQ8, 96], bf16, tag="tps")
                        nc.sync.dma_start_transpose(tps, pv_sb[:, :])
                        rec = recsp.tile([P, NQ8, 1], f32, tag="rec")
                        nc.vector.reciprocal(rec, tps[:, :, DH:DH + 1])
                        col = hp_p * P + st * DH
                        o_ap = out_sb[:, qb_p * NQ8:(qb_p + 1) * NQ8,
                                      col:col + DH]
                        in0 = tps[:, :, :DH]
                        in0b, in1b = broadcast_tensor_aps(in0, rec)
                        nc.vector.tensor_tensor(
                            out=o_ap, in0=in0b, in1=in1b,
                            op=mybir.AluOpType.mult,
                        )
                    nc.sync.dma_start(
                        out_blk[:, qb_p * NQ8:(qb_p + 1) * NQ8,
                                hp_p * P:(hp_p + 1) * P],
                        out_sb[:, qb_p * NQ8:(qb_p + 1) * NQ8,
                               hp_p * P:(hp_p + 1) * P],
                    )

                blocks = [(0, 0), (0, 1), (1, 0), (1, 1)]

                with tc.tile_pool(name="pp", bufs=2, space="PSUM") as pp:
                    # upfront projections (fp8 DoubleRow): all of K (its DMA
                    # lands first), then the qb=0 half of Q
                    for hp in range(2):
                        for co, cw in _chunks(nk, CH):
                            proj_qk_group("k", kx_sb, wk_sb, KT, hp, co, cw)
                    for hp in range(2):
                        for c in range(NCH):
                            proj_qk_group("q", qx_sb, wq_sb, QT,
                                          hp, c * CH, CH)
                    # qb=1 Q-projection groups ride block 0's later units
                    # (their qx DMA lands last); V projection rides every unit
                    qb1 = [
                        (lambda hp=hp, c=c: proj_qk_group(
                            "q", qx_sb, wq_sb, QT, hp, QB + c * CH, CH))
                        for hp in range(2) for c in range(NCH)
                    ]
                    ets0 = emit_block(*blocks[0], weave_pv=None, weave_v=True,
                                      extra_weave=qb1)

                with tc.tile_pool(name="pvt", bufs=2, space="PSUM") as pvtp:
                    prev = None
                    prev_ets = ets0
                    prev_blk = blocks[0]
                    for bi in range(1, 4):
                        hp, qb = blocks[bi]
                        pvts = [
                            pvtp.tile([DH + 1, QB], f32, tag="pvt",
                                      name=f"pvt_{bi}_{st}")
                            for st in range(2)
                        ]
                        pv_state = (pvts, prev_ets, prev_blk[0], prev_blk[1])
                        ets = emit_block(hp, qb, weave_pv=pv_state,
                                         weave_v=False)
                        emit_normalize(pv_state)
                        prev_ets = ets
                        prev_blk = blocks[bi]
                    # tail: last block's P@V + normalize
                    pvts = [
                        pvtp.tile([DH + 1, QB], f32, tag="pvt",
                                  name=f"pvt_tail_{st}")
                        for st in range(2)
                    ]
                    pv_state = (pvts, prev_ets, prev_blk[0], prev_blk[1])
                    for kt in range(NTK):
                        emit_pv_unit(pv_state, kt)
                    emit_normalize(pv_state)
    nc.compile()
    return nc


def _get_nc(nk, use_bias=False):
    key = (nk, use_bias)
    if key not in _CACHE:
        _CACHE[key] = _build_nc(nk, use_bias=use_bias)
    return _CACHE[key]


def _run(nc, in_maps, trace=False):
    from concourse.bass_utils import run_bass_kernel_spmd

    return run_bass_kernel_spmd(
        nc, in_maps, core_ids=list(range(NCORES)), trace=trace
    )


def _make_in_maps(q, k, v, mask, Wq, bq, Wk, bk, Wv, bv):
    import ml_dtypes

    bf16 = ml_dtypes.bfloat16
    fp8 = ml_dtypes.float8_e4m3fn
    q = np.asarray(q, np.float32)
    k = np.asarray(k, np.float32)
    v = np.asarray(v, np.float32)
    mask = np.asarray(mask, np.int32)
    Wq = np.asarray(Wq, np.float32)
    Wk = np.asarray(Wk, np.float32)
    Wv = np.asarray(Wv, np.float32)

    use_bias = bool(
        np.any(np.asarray(bq, np.float32))
        or np.any(np.asarray(bk, np.float32))
        or np.any(np.asarray(bv, np.float32))
    )

    idxs = [np.nonzero(mask[b])[0] for b in range(B)]
    neff = max(1, max(len(ix) for ix in idxs))
    nk = -(-neff // P) * P  # round up to multiple of 128

    def pair4(x):  # [D, w] -> [P, ND2, 2, w] fp8 (d = d2*256 + ko*128 + p)
        w = x.shape[1]
        return np.ascontiguousarray(
            x.reshape(ND2, 2, P, w).transpose(2, 0, 1, 3)
        ).astype(fp8)

    def tile8(x):  # [D, w] -> [P, ND, w]
        w = x.shape[1]
        return np.ascontiguousarray(x.reshape(ND, P, w).transpose(1, 0, 2))

    qxs, kxs, vxs, mks = [], [], [], []
    for b in range(B):
        ix = idxs[b]
        qxs.append(pair4(q[b].T))
        kc = np.zeros((D, nk), np.float32)
        vc = np.zeros((D, nk), np.float32)
        kc[:, :len(ix)] = k[b].T[:, ix]
        vc[:, :len(ix)] = v[b].T[:, ix]
        kxs.append(pair4(kc))
        vxs.append(tile8(vc).astype(bf16))
        m = np.zeros((nk,), np.int32)
        m[:len(ix)] = 1
        mks.append(m)

    in_maps = []
    for c in range(NCORES):
        b, g = divmod(c, GROUPS)
        cols = slice(g * GW, (g + 1) * GW)
        im = {
            "qx": qxs[b],
            "kx": kxs[b],
            "vx": vxs[b],
            "wq": pair4(Wq[:, cols]),
            "wk": pair4(Wk[:, cols]),
            "wv": tile8(Wv[:, cols]).astype(bf16),
            "mask": mks[b],
        }
        if use_bias:
            im["bq"] = np.ascontiguousarray(bq[cols]).astype(bf16)
            im["bk"] = np.ascontiguousarray(bk[cols]).astype(bf16)
            im["bv"] = np.ascontiguousarray(bv[cols]).astype(bf16)
        in_maps.append(im)
    return nk, use_bias, in_maps


def _assemble(results):
    out = np.empty((B, S, D), np.float32)
    for c in range(NCORES):
        b, g = divmod(c, GROUPS)
        out[b, :, g * GW:(g + 1) * GW] = results[c]["out"].astype(np.float32)
    return out


def kernel(q, k, v, mask, Wq, bq, Wk, bk, Wv, bv):
    nk, use_bias, in_maps = _make_in_maps(q, k, v, mask, Wq, bq, Wk, bk, Wv, bv)
    res = _run(_get_nc(nk, use_bias), in_maps, trace=False)
    return _assemble(res.results)


def _install_ntff_hook():
    """The image's antenv stub lacks axon_hooks; synthesize it and register
    the ctypes NTFF hook that trn_agent_boot would have installed."""
    import sys
    import types

    import antenv

    if "antenv.axon_hooks" in sys.modules:
        return
    mod = types.ModuleType("antenv.axon_hooks")
    state = {"hook": None}
    mod.set_axon_ntff_profile_hook = lambda h: state.__setitem__("hook", h)
    mod.get_axon_ntff_profile_hook = lambda: state["hook"]
    sys.modules["antenv.axon_hooks"] = mod
    antenv.axon_hooks = mod
    try:
        from trn_agent_boot.trn_boot import _ntff_profile_via_ctypes

        mod.set_axon_ntff_profile_hook(
            _ntff_profile_via_ctypes("/opt/axon/libaxon_pjrt.so")
        )
    except Exception as e:
        print(f"ntff hook registration failed: {e}")


def _exec_ns_from_newest_ntff():
    """Span of the newest NTFF json's DMA events — matches gauge's
    first/last-useful exec time when instruction events are absent."""
    import glob
    import json as _json
    import os

    try:
        path = max(glob.glob("/tmp/tmp*/ntff_0.json"), key=os.path.getmtime)
        d = _json.load(open(path))
        ev = d.get("dma", [])
        if not ev:
            return None
        t0 = min(e["timestamp"] for e in ev)
        t1 = max(e["timestamp"] + e.get("duration", 0) for e in ev)
        return t1 - t0
    except Exception:
        return None


def kernel_traced(q, k, v, mask, Wq, bq, Wk, bk, Wv, bv):
    """Same as kernel() but also returns (output, exec_time_ns)."""
    _install_ntff_hook()
    nk, use_bias, in_maps = _make_in_maps(q, k, v, mask, Wq, bq, Wk, bk, Wv, bv)
    nc = _get_nc(nk, use_bias)
    try:
        res = _run(nc, in_maps, trace=True)
        return _assemble(res.results), res.exec_time_ns
    except Exception:
        # gauge's NTFF->perfetto step can fail on kernels whose profile
        # lacks instruction events (`assert insts`); the NTFF json still
        # exists, so recover the exec time from its DMA span and rerun
        # untraced for the outputs.
        exec_ns = _exec_ns_from_newest_ntff()
        res = _run(nc, in_maps, trace=False)
        return _assemble(res.results), exec_ns
